# revision 8
# baseline (speedup 1.0000x reference)
"""BitLinear (2-bit ternary packed weights) Trainium2 Bass kernel.

Full-input contract: kernel(x, weight, weight_scale) -> (2, 2048, 12288) f32.
Tensor-parallel over 8 NeuronCores: weight rows (out_features) sharded
8 x 1536, x replicated, outputs concatenated host-side.

Math notes:
  - reference: x_i8 = round(x * 127/absmax_tok); W = unpack2bit(weight)-1
    out = (x_i8 @ W.T) * weight_scale * absmax_tok/127
  - we matmul against the raw 2-bit codes c in {0,1,2,3} (bf16-exact) and
    fold the -1 via out_int = psum - rowsum(x_i8).
  - bf16 holds integers up to 256 exactly; f32 PSUM accumulation of
    integer products stays < 2^24 => matmul is bit-exact vs f32 reference.
  - rounding uses the +/- 1.5*2^23 magic trick == round-half-to-even
    (matches jnp.round). clip(-128,127) is a provable no-op since
    |x*q| <= 127*(1+eps).
"""

import os
from contextlib import ExitStack

import numpy as np

import concourse.bass as bass
import concourse.mybir as mybir
import concourse.tile as tile
from concourse import bacc
from concourse.bass import ds, ts
from concourse.bass_utils import run_bass_kernel_spmd
from concourse.masks import make_identity

# problem shapes (hardcoded per contract)
B, T, K, M = 2, 2048, 4096, 12288
N = B * T
N_CORES = 8
M_CORE = M // N_CORES

MAGIC = 12582912.0  # 1.5 * 2**23: add+sub forces RNE rounding to integer

f32 = mybir.dt.float32
bf16 = mybir.dt.bfloat16
i32 = mybir.dt.int32
Alu = mybir.AluOpType
Act = mybir.ActivationFunctionType
Ax = mybir.AxisListType


def declare_io(nc: bass.Bass, n: int, k: int, m_core: int):
    x = nc.dram_tensor("x", [n, k], f32, kind="ExternalInput").ap()
    wp = nc.dram_tensor("wp", [m_core, k // 4], i32, kind="ExternalInput").ap()
    ws = nc.dram_tensor("ws", [1], f32, kind="ExternalInput").ap()
    out = nc.dram_tensor("out", [n, m_core], f32, kind="ExternalOutput").ap()
    return x, wp, ws, out


def emit(tc: tile.TileContext, ctx: ExitStack, aps, n: int, k: int, m_core: int):
    nc = tc.nc
    x, wp, ws, out = aps
    assert n % 128 == 0 and k % 512 == 0 and m_core % 128 == 0
    KT = k // 128  # number of 128-wide k tiles (== number of packed 32B blocks)
    NT = n // 128
    MC = 512 if m_core % 512 == 0 else m_core
    assert m_core % MC == 0
    NMC = m_core // MC

    const = ctx.enter_context(tc.tile_pool(name="const", bufs=1))
    ident = const.tile([128, 128], bf16)
    make_identity(nc, ident[:])
    c127 = const.tile([128, 1], f32)
    nc.vector.memset(c127[:], 127.0)
    c1 = const.tile([128, 1], f32)
    nc.vector.memset(c1[:], 1.0)
    wsb = const.tile([128, 1], f32)
    nc.gpsimd.dma_start(out=wsb[:], in_=ws.to_broadcast((128, 1)))

    wt_pool = ctx.enter_context(tc.tile_pool(name="wt", bufs=1))
    WT = wt_pool.tile([128, KT, m_core], bf16)  # [k%128, ktile, m] codes {0..3}

    # ---------------- weight unpack prologue ----------------
    # packed byte b of row m (stored as one int32 each, value 0..255):
    # block = b//32, j = b%32; weight k = block*128 + g*32 + j uses shift 6-2g.
    # Pools stay alive for the whole kernel: letting them be reused by the
    # main loop creates cross-phase WAR deps that exceed the HW DMA
    # wait-slot limit. Half-K staging keeps them small enough to coexist.
    KH = KT // 2 if KT % 2 == 0 else KT  # ktiles per staging chunk
    NH = KT // KH
    pk_pool = ctx.enter_context(tc.tile_pool(name="pk", bufs=2))
    wi_pool = ctx.enter_context(tc.tile_pool(name="wi", bufs=2))
    wn_pool = ctx.enter_context(tc.tile_pool(name="wn", bufs=2))
    tpw_pool = ctx.enter_context(tc.tile_pool(name="tpw", bufs=2, space="PSUM"))
    for mi in range(m_core // 128):
        pkt = pk_pool.tile([128, k // 4], i32)
        nc.gpsimd.dma_start(out=pkt[:], in_=wp[ts(mi, 128), :])
        pk_v = pkt[:].rearrange("p (b j) -> p b j", j=32)
        for h in range(NH):
            # bitVec ops cannot cast: extract to int32, then convert to bf16
            wn_i = wi_pool.tile([128, KH, 4, 32], i32)  # [m, ktile, g, j]
            for g in range(4):
                nc.vector.tensor_scalar(
                    out=wn_i[:, :, g, :],
                    in0=pk_v[:, ds(h * KH, KH), :],
                    scalar1=6 - 2 * g,
                    scalar2=3,
                    op0=Alu.logical_shift_right,
                    op1=Alu.bitwise_and,
                )
            wn = wn_pool.tile([128, KH, 4, 32], bf16)
            nc.vector.tensor_copy(
                wn[:].rearrange("p t g j -> p (t g j)"),
                wn_i[:].rearrange("p t g j -> p (t g j)"),
            )
            wn_v = wn[:].rearrange("p t g j -> p t (g j)")
            for tq in range((KH + 3) // 4):
                nt = min(4, KH - tq * 4)
                tp = tpw_pool.tile([128, 512], bf16)
                for j in range(nt):
                    t = tq * 4 + j
                    nc.tensor.transpose(tp[:, ts(j, 128)], wn_v[:, t, :], ident[:])
                nc.scalar.copy(
                    out=WT[:, ds(h * KH + tq * 4, nt), ds(mi * 128, 128)],
                    in_=tp[:, ds(0, nt * 128)].rearrange("p (a b) -> p a b", b=128),
                )

    # ---------------- main loop over token tiles ----------------
    xf_pool = ctx.enter_context(tc.tile_pool(name="xf", bufs=2))
    xq_pool = ctx.enter_context(tc.tile_pool(name="xq", bufs=2))
    xt_pool = ctx.enter_context(tc.tile_pool(name="xt", bufs=2))
    sm_pool = ctx.enter_context(tc.tile_pool(name="sm", bufs=3))
    ob_pool = ctx.enter_context(tc.tile_pool(name="ob", bufs=3))
    tpx_pool = ctx.enter_context(tc.tile_pool(name="tpx", bufs=2, space="PSUM"))
    mm_pool = ctx.enter_context(tc.tile_pool(name="mm", bufs=3, space="PSUM"))

    for ni in range(NT):
        xf = xf_pool.tile([128, k], f32)
        nc.gpsimd.dma_start(out=xf[:], in_=x[ts(ni, 128), :])

        am = sm_pool.tile([128, 1], f32)
        nc.vector.tensor_reduce(
            am[:], xf[:], axis=Ax.X, op=Alu.max, apply_absolute_value=True
        )
        am2 = sm_pool.tile([128, 1], f32)
        nc.vector.tensor_scalar_max(am2[:], am[:], 1e-5)
        r = sm_pool.tile([128, 1], f32)  # 1/absmax (HW iterative divide)
        nc.vector.reciprocal(r[:], am2[:])
        q = sm_pool.tile([128, 1], f32)  # 127/absmax (to ~1ulp of reference)
        nc.vector.tensor_scalar_mul(q[:], r[:], 127.0)
        a = sm_pool.tile([128, 1], f32)  # act_scale = 1/q
        nc.vector.reciprocal(a[:], q[:])
        s = sm_pool.tile([128, 1], f32)  # act_scale * weight_scale
        nc.vector.tensor_tensor(s[:], a[:], wsb[:], op=Alu.mult)

        # x_i8 = RNE(x*q): in-place (x*q + MAGIC), then (- MAGIC) -> bf16
        nc.vector.tensor_scalar(
            out=xf[:], in0=xf[:], scalar1=q[:], scalar2=MAGIC,
            op0=Alu.mult, op1=Alu.add,
        )
        xq = xq_pool.tile([128, k], bf16)
        nc.vector.tensor_scalar(
            out=xq[:], in0=xf[:], scalar1=MAGIC, scalar2=None, op0=Alu.subtract
        )
        rs = sm_pool.tile([128, 1], f32)  # rowsum(x_i8) for the code-1 fold
        nc.vector.tensor_reduce(rs[:], xq[:], axis=Ax.X, op=Alu.add)
        nrss = sm_pool.tile([128, 1], f32)  # -rowsum * s  (bias for dequant)
        nc.vector.tensor_scalar(
            out=nrss[:], in0=rs[:], scalar1=s[:], scalar2=-1.0,
            op0=Alu.mult, op1=Alu.mult,
        )

        # transpose x_i8 -> xT [k%128, ktile, n%128]
        xT = xt_pool.tile([128, KT, 128], bf16)
        xq_v = xq[:].rearrange("p (t j) -> p t j", j=128)
        for tq in range((KT + 3) // 4):
            nt = min(4, KT - tq * 4)
            tp = tpx_pool.tile([128, 512], bf16)
            for j in range(nt):
                t = tq * 4 + j
                nc.tensor.transpose(tp[:, ts(j, 128)], xq_v[:, t, :], ident[:])
            nc.scalar.copy(
                out=xT[:, ds(tq * 4, nt), :],
                in_=tp[:, ds(0, nt * 128)].rearrange("p (a b) -> p a b", b=128),
            )

        for mc in range(NMC):
            ps = mm_pool.tile([128, MC], f32)
            for t in range(KT):
                nc.tensor.matmul(
                    ps[:],
                    lhsT=xT[:, t, :],
                    rhs=WT[:, t, ds(mc * MC, MC)],
                    start=(t == 0),
                    stop=(t == KT - 1),
                )
            ob = ob_pool.tile([128, MC], f32)
            # out = psum*s + (-rowsum*s)  == (psum - rowsum) * s
            nc.scalar.activation(
                ob[:], ps[:], Act.Identity, bias=nrss[:], scale=s[:]
            )
            nc.gpsimd.dma_start(out=out[ts(ni, 128), ds(mc * MC, MC)], in_=ob[:])


def build(n: int = N, k: int = K, m_core: int = M_CORE, num_devices: int = N_CORES):
    nc = bacc.Bacc(
        "TRN2", target_bir_lowering=False, debug=False, num_devices=num_devices
    )
    aps = declare_io(nc, n, k, m_core)
    with tile.TileContext(nc) as tc:
        with ExitStack() as ctx:
            emit(tc, ctx, aps, n, k, m_core)
    nc.compile()
    return nc


_CACHE: dict = {}


def kernel(x: np.ndarray, weight: np.ndarray, weight_scale: np.ndarray) -> np.ndarray:
    orig_shape = x.shape
    x2 = np.ascontiguousarray(x.reshape(-1, orig_shape[-1]), dtype=np.float32)
    w = np.ascontiguousarray(weight, dtype=np.int32)
    ws = np.ascontiguousarray(weight_scale, dtype=np.float32)
    assert x2.shape == (N, K) and w.shape == (M, K // 4)

    if "nc" not in _CACHE:
        _CACHE["nc"] = build()
    nc = _CACHE["nc"]

    in_maps = [
        {"x": x2, "wp": w[i * M_CORE : (i + 1) * M_CORE], "ws": ws}
        for i in range(N_CORES)
    ]
    res = run_bass_kernel_spmd(
        nc,
        in_maps,
        list(range(N_CORES)),
        trace=bool(int(os.environ.get("BITLINEAR_TRACE", "0"))),
    )
    _CACHE["last_result"] = res
    outs = [res.results[i]["out"] for i in range(N_CORES)]
    full = np.concatenate(outs, axis=1)
    return full.reshape(*orig_shape[:-1], M).astype(x.dtype, copy=False)
